# revision 1
# baseline (speedup 1.0000x reference)
"""DigitCapsules routing kernel for 8 Trainium2 NeuronCores.

Strategy: shard the in_capsule dimension (32 -> 4 per core) so each core
reads only its slice of W (the dominant tensor) and u.  Each core computes
its u_sum slice [B, 4, OC, OCH] and u_dot slice [4, OC] with TensorE
matmuls (bf16 inputs, fp32 accumulate), one AllGather shares all slices,
and every core then runs the tiny 3-iteration dynamic-routing loop
redundantly on the full u_sum.  Core 0's output is returned.

Layouts: the gathered u_sum lives as U_G[p, r, (b c)] with the partition
index p = 16*ia + 4*ib + j encoding (rank ia, local capsule ib, o-group j)
and o = 4*r + j.  The per-o mixing weights live in a zero-masked lhsT
cgz[p, 10*r + o] = c[i(p), o] * [o == 4*r + j(p)], so a plain k=128 matmul
per (r, half) computes s_j; the masking zeros kill the wrong-o terms.

Self-contained: hardcodes all shapes; only imports installed packages.
"""

import numpy as np

# problem shapes
B = 64
IC = 32
ICH = 8
WID = 6
HEI = 6
D = ICH * WID * HEI          # 288
DP = 384                     # D padded to 3 k-chunks of 128
KC = 3                       # k-chunks per contraction
OC = 10
OCH = 16
BC = B * OCH                 # 1024
NUM_ROUTING = 3
NCORES = 8
ICL = IC // NCORES           # 4 local in-capsules per core

# AllGather chunk layout per rank (bf16): (ib:4, j:4, r:3, b:64, c:16)
# with o = 4*r + j.  Slots with o >= 10 are zero padding that the cgz mask
# ignores, so u_dot (ib, o) is stashed inside the (ib=0, j=2, r=2) pad slot
# and the whole chunk stays contiguous (one-DMA gather on the far side).
USZ = 4 * 4 * 3 * B * OCH    # 49152 bf16 -> 98304 B, 32-aligned
CHUNK = USZ
UD_OFF = 2 * 3 * B * OCH + 2 * B * OCH  # 8192, slot (ib=0, j=2, r=2)

_CACHE = {}


def build(reps=1, single_core=False):
    """Build + compile the SPMD Bass program (cached per reps).

    reps > 1 repeats the complete pipeline sequentially; used only for
    wall-clock differencing to estimate the on-device execution time.
    single_core=True swaps the AllGather for local DMA copies (same bytes
    moved) so the collective-free program can run under TimelineSim.
    """
    key = ("nc", reps, single_core)
    if key in _CACHE:
        return _CACHE[key]

    import concourse.bass as bass  # noqa: F401
    import concourse.mybir as mybir
    from concourse import tile, bacc

    f32 = mybir.dt.float32
    bf16 = mybir.dt.bfloat16
    AX = mybir.AxisListType
    OP = mybir.AluOpType
    ACT = mybir.ActivationFunctionType

    nc = bacc.Bacc("TRN2", target_bir_lowering=False, debug=False,
                   num_devices=1 if single_core else NCORES)

    # inputs pre-arranged on the host so every load is a contiguous DMA
    u_in = nc.dram_tensor("u_t", [128, ICL, KC, B], bf16, kind="ExternalInput")
    w_in = nc.dram_tensor("w_t", [128, ICL, KC, OC * OCH], bf16,
                          kind="ExternalInput")
    rep_in = nc.dram_tensor("repm", [32, 128], f32, kind="ExternalInput")
    mask_in = nc.dram_tensor("maskm", [128, 3 * OC], bf16,
                             kind="ExternalInput")
    v_out = nc.dram_tensor("v", [B, OC, OCH], f32, kind="ExternalOutput")

    def emit(tc, sb, dram):
        # ---------------- phase 1: local u_sum + u_dot ----------------
        u_sb = sb.tile([128, ICL, KC, B], bf16, name="u_sb")
        nc.sync.dma_start(u_sb[:], u_in[:])
        w_sb = sb.tile([128, ICL, KC, OC * OCH], bf16, name="w_sb")
        nc.sync.dma_start(w_sb[:], w_in[:])
        rep_sb = sb.tile([32, 128], f32, name="rep_sb")
        nc.sync.dma_start(rep_sb[:], rep_in[:])
        mask_sb = sb.tile([128, 3 * OC], bf16, name="mask_sb")
        nc.sync.dma_start(mask_sb[:], mask_in[:])

        # us[d] = sum_b u  (for u_dot); reduce in fp32 then snap to bf16
        us_f = sb.tile([128, ICL, KC], f32, name="us_f")
        for i in range(ICL):
            nc.vector.tensor_reduce(
                us_f[:, i, :], u_sb[:, i, :, :], axis=AX.X, op=OP.add)
        us_b = sb.tile([128, ICL, KC], bf16, name="us_b")
        nc.scalar.activation(us_b[:], us_f[:], ACT.Copy)

        cc_in = dram.tile([CHUNK], bf16, name="cc_in")
        # u_sum chunk view for stores: (ib, j, r, b, c) -> (ib, b, jr, c)
        cc_us = cc_in[0:USZ].rearrange(
            "(ib jr b c) -> ib b jr c", ib=ICL, jr=12, b=B, c=OCH)

        ud_f = sb.tile([1, ICL * OC], f32, name="ud_f")
        with tc.tile_pool(name="ps1", bufs=2, space="PSUM") as ps1:
            for i in range(ICL):
                p1 = ps1.tile([B, OC, OCH], f32, name="p1")
                for k in range(KC):
                    nc.tensor.matmul(
                        p1[:], u_sb[:, i, k, :], w_sb[:, i, k, :],
                        start=(k == 0), stop=(k == KC - 1))
                # s1 columns are (j, r) with o = 4r + j; (j>=2, r=2) is pad
                s1 = sb.tile([B, 4, 3, OCH], bf16, name="s1", bufs=2)
                nc.gpsimd.memset(s1[:, 2:4, 2, :], 0.0)
                nc.vector.tensor_copy(
                    s1[:].rearrange("b j r c -> b r j c")[:, 0:2],
                    p1[:, 0:8, :].rearrange("b (r j) c -> b r j c", r=2, j=4))
                nc.scalar.activation(s1[:, 0:2, 2, :], p1[:, 8:10, :], ACT.Copy)
                nc.sync.dma_start(cc_us[i], s1[:].rearrange(
                    "b j r c -> b (j r) c"))

                # u_dot row i: sum_d us[d] * W[d, (o c)], then sum over c
                udp = ps1.tile([1, OC * OCH], f32, name="udp")
                for k in range(KC):
                    nc.tensor.matmul(
                        udp[:], us_b[:, i, k:k + 1], w_sb[:, i, k, :],
                        start=(k == 0), stop=(k == KC - 1))
                nc.vector.tensor_reduce(
                    ud_f[:].rearrange("p (o i) -> p i o", o=OC, i=ICL)[:, i],
                    udp[:].rearrange("p (o c) -> p o c", c=OCH),
                    axis=AX.X, op=OP.add)
            ud_sb = sb.tile([1, ICL * OC], bf16, name="ud_sb")
            nc.scalar.activation(ud_sb[:], ud_f[:], ACT.Copy)
            nc.sync.dma_start(cc_in[UD_OFF:UD_OFF + ICL * OC], ud_sb[:])

        # ---------------- AllGather ----------------
        cc_out = dram.tile([NCORES * CHUNK], bf16,
                           addr_space="Local" if single_core else "Shared",
                           name="cc_out")
        if single_core:
            for ia in range(NCORES):
                nc.sync.dma_start(
                    cc_out[ia * CHUNK:(ia + 1) * CHUNK], cc_in[:])
        else:
            nc.gpsimd.collective_compute(
                "AllGather", OP.bypass,
                replica_groups=[list(range(NCORES))],
                ins=[cc_in[:]], outs=[cc_out[:]],
            )

        # ---------------- load gathered tensors ----------------
        # U_G[p = 16 ia + 4 ib + j, r, (b c)]: the chunks are contiguous,
        # so the whole gather loads in one full-width DMA
        U_G = sb.tile([128, 3, BC], bf16, name="U_G")
        nc.sync.dma_start(
            U_G[:],
            cc_out[:].rearrange("(p r f) -> p r f", p=128, r=3, f=BC))

        # u_dot in [o, (ia, ib)] layout; the [10, 8, 8] tile keeps the
        # (ia, ib) dims non-collapsible so the DMA AP balances 1:1
        ud_t2 = sb.tile([OC, NCORES, 8], bf16, name="ud_t2")
        cc_r = cc_out[:].rearrange("(a x) -> a x", a=NCORES)
        nc.sync.dma_start(
            ud_t2[:, :, 0:ICL],
            cc_r[:, UD_OFF:UD_OFF + ICL * OC].rearrange(
                "a (o i) -> o a i", i=ICL, o=OC))

        # ---------------- routing loop ----------------
        ones_kk = sb.tile([OC, OC], f32, name="ones_kk")
        nc.vector.memset(ones_kk[:], 1.0)

        ct32 = sb.tile([32, 32], f32, name="ct32")
        nc.vector.memset(ct32[:], 0.0)
        cit = sb.tile([32, 32], f32, name="cit")
        bt = sb.tile([OC, IC], f32, name="bt")
        nc.vector.memset(bt[:], 0.0)

        with tc.tile_pool(name="ps2", bufs=1, space="PSUM") as ps2:
            for t in range(NUM_ROUTING):
                # c = softmax_i(b)  computed in [o, i] layout
                esum = sb.tile([OC, 1], f32, name="esum")
                nc.scalar.activation(ct32[0:OC, :], bt[:], ACT.Exp,
                                     accum_out=esum[:])
                # exp stays unnormalized; 1/sumexp rides the post-matmul
                # per-partition scalars instead
                nc.vector.transpose(cit[:], ct32[:])

                # CEXP[p, o] = c[i(p), o] via one replication matmul,
                # then cgz = CEXP (broadcast over r) * mask
                cexp = ps2.tile([128, 1, OC], f32, name="cexp")
                nc.tensor.matmul(cexp[:, 0, :], rep_sb[:], cit[:, 0:OC],
                                 start=True, stop=True)
                cgz = sb.tile([128, 3 * OC], bf16, name="cgz")
                nc.vector.tensor_tensor(
                    cgz[:].rearrange("p (r o) -> p r o", r=3, o=OC),
                    cexp[:].broadcast_to([128, 3, OC]),
                    mask_sb[:].rearrange("p (r o) -> p r o", r=3, o=OC),
                    OP.mult)

                # 1/sumexp, emitted after cgz so it never delays the
                # matmul-gating DVE chain
                rcp = sb.tile([OC, 1], f32, name="rcp")
                nc.vector.reciprocal_approx_fast(rcp[:], esum[:])

                # s_j[o, (b c)] = sum_i c[i, o] * u_sum[i, b, o, c]
                sj_ps = ps2.tile([OC, BC], f32, name="sj_ps")
                for h in range(2):
                    for r in range(3):
                        nc.tensor.matmul(
                            sj_ps[:, 512 * h:512 * (h + 1)],
                            cgz[:, OC * r:OC * (r + 1)],
                            U_G[:, r, 512 * h:512 * (h + 1)],
                            start=(r == 0), stop=(r == 2))

                # n = sum |s| ; srow[o] = sum_(b,c) s
                abs_scr = sb.tile([OC, BC], bf16, name="abs_scr")
                absraw = sb.tile([OC, 1], f32, name="absraw")
                nc.scalar.activation(abs_scr[:], sj_ps[:], ACT.Abs,
                                     accum_out=absraw[:])
                absrow = sb.tile([OC, 1], f32, name="absrow")
                nc.vector.tensor_scalar(absrow[:], absraw[:], rcp[:], None,
                                        OP.mult)
                # nbc[o] = n on every partition (all-ones matmul broadcast)
                nbc = ps2.tile([OC, 1], f32, name="nbc")
                nc.tensor.matmul(nbc[:], ones_kk[:], absrow[:],
                                 start=True, stop=True)
                # scale10 = rcp * n / (1 + n^2), per partition
                dd = sb.tile([OC, 1], f32, name="dd")
                nc.vector.tensor_scalar(dd[:], nbc[:], nbc[:], 1.0,
                                        OP.mult, OP.add)
                rdd = sb.tile([OC, 1], f32, name="rdd")
                nc.vector.reciprocal_approx_fast(rdd[:], dd[:])
                scale10 = sb.tile([OC, 1], f32, name="scale10")
                nc.vector.tensor_scalar(scale10[:], rdd[:], nbc[:], rcp[:],
                                        OP.mult, OP.mult)

                if t < NUM_ROUTING - 1:
                    srow = sb.tile([OC, 1], f32, name="srow")
                    nc.vector.tensor_reduce(srow[:], sj_ps[:], axis=AX.X,
                                            op=OP.add)
                    # b += u_dot * (scale * srow)[o]
                    agree = sb.tile([OC, IC], f32, name="agree")
                    nc.vector.tensor_scalar(agree[:], ud_t2[:, :, 0:ICL], srow[:],
                                            scale10[:], OP.mult, OP.mult)
                    nc.vector.tensor_tensor(bt[:], bt[:], agree[:], OP.add)
                else:
                    v_sb = sb.tile([OC, BC], f32, name="v_sb")
                    nc.vector.tensor_scalar(v_sb[:], sj_ps[:], scale10[:],
                                            None, OP.mult)
                    nc.sync.dma_start(
                        v_out[:].rearrange("b o c -> o b c"),
                        v_sb[:].rearrange("o (b c) -> o b c", b=B, c=OCH))

    with tile.TileContext(nc) as tc:
        with (
            tc.tile_pool(name="sb", bufs=1) as sb,
            tc.tile_pool(name="dram", bufs=1, space="DRAM") as dram,
        ):
            for _rep in range(reps):
                emit(tc, sb, dram)

    nc.compile()
    _CACHE[key] = nc
    return nc


def make_in_maps(u, W):
    import ml_dtypes
    bf16 = ml_dtypes.bfloat16

    u = np.ascontiguousarray(np.asarray(u, dtype=np.float32))
    W = np.ascontiguousarray(np.asarray(W, dtype=np.float32))
    # [B, IC, D] -> per-core [p:128, i:4, k:3, b/f] contiguous layouts
    u_t = np.zeros((IC, KC, 128, B), dtype=np.float32)
    u_t.reshape(IC, DP, B)[:, :D, :] = u.reshape(B, IC, D).transpose(1, 2, 0)
    u_t = np.ascontiguousarray(u_t.transpose(2, 0, 1, 3)).astype(bf16)
    w_t = np.zeros((IC, KC, 128, OC * OCH), dtype=np.float32)
    w_t.reshape(IC, DP, OC * OCH)[:, :D, :] = W.reshape(IC, D, OC * OCH)
    w_t = np.ascontiguousarray(w_t.transpose(2, 0, 1, 3)).astype(bf16)

    # repm[i, p] = [i == p // 4]; maskm[p, 10r + o] = [o == 4r + (p % 4)]
    p = np.arange(128)
    repm = (np.arange(32)[:, None] == (p[None, :] // 4)).astype(np.float32)
    maskm = np.zeros((128, 3 * OC), dtype=np.float32)
    for r in range(3):
        for o in range(OC):
            maskm[:, OC * r + o] = (o == 4 * r + (p % 4))
    maskm = maskm.astype(bf16)

    return [
        {
            "u_t": np.ascontiguousarray(u_t[:, ICL * r: ICL * (r + 1)]),
            "w_t": np.ascontiguousarray(w_t[:, ICL * r: ICL * (r + 1)]),
            "repm": repm,
            "maskm": maskm,
        }
        for r in range(NCORES)
    ]


def get_runner(nc):
    """Build (once) a jitted 8-core executor for the compiled program.

    Mirrors bass2jax.run_bass_via_pjrt's multi-core path but caches the
    jitted callable so repeated kernel() calls skip retracing.
    """
    if "runner" in _CACHE and _CACHE["runner"][0] is nc:
        return _CACHE["runner"][1]

    import jax
    from jax.sharding import Mesh, PartitionSpec
    from jax.experimental.shard_map import shard_map
    from concourse import mybir
    from concourse.bass2jax import (_bass_exec_p, install_neuronx_cc_hook,
                                    partition_id_tensor)

    install_neuronx_cc_hook()
    partition_name = (nc.partition_id_tensor.name
                      if nc.partition_id_tensor else None)
    in_names, out_names, out_avals, zero_outs = [], [], [], []
    for alloc in nc.m.functions[0].allocations:
        if not isinstance(alloc, mybir.MemoryLocationSet):
            continue
        name = alloc.memorylocations[0].name
        if alloc.kind == "ExternalInput":
            if name != partition_name:
                in_names.append(name)
        elif alloc.kind == "ExternalOutput":
            out_names.append(name)
            shape = tuple(alloc.tensor_shape)
            dtype = mybir.dt.np(alloc.dtype)
            out_avals.append(jax.core.ShapedArray(shape, dtype))
            zero_outs.append(np.zeros(shape, dtype))
    n_params = len(in_names)
    n_outs = len(out_avals)
    all_in_names = list(in_names) + list(out_names)
    if partition_name is not None:
        all_in_names.append(partition_name)

    def _body(*args):
        operands = list(args)
        if partition_name is not None:
            operands.append(partition_id_tensor())
        return tuple(_bass_exec_p.bind(
            *operands,
            out_avals=tuple(out_avals),
            in_names=tuple(all_in_names),
            out_names=tuple(out_names),
            lowering_input_output_aliases=(),
            sim_require_finite=True,
            sim_require_nnan=True,
            nc=nc,
        ))

    devices = jax.devices()[:NCORES]
    mesh = Mesh(np.asarray(devices), ("core",))
    sharded = jax.jit(
        shard_map(_body, mesh=mesh,
                  in_specs=(PartitionSpec("core"),) * (n_params + n_outs),
                  out_specs=(PartitionSpec("core"),) * n_outs,
                  check_rep=False),
        donate_argnums=tuple(range(n_params, n_params + n_outs)),
        keep_unused=True)

    def run(in_maps):
        concat_in = [
            np.concatenate([np.asarray(m[nm]) for m in in_maps], axis=0)
            for nm in in_names
        ]
        concat_zeros = [np.zeros((NCORES * z.shape[0], *z.shape[1:]), z.dtype)
                        for z in zero_outs]
        outs = sharded(*concat_in, *concat_zeros)
        jax.block_until_ready(outs)
        return {
            nm: np.asarray(outs[i]).reshape(NCORES, *out_avals[i].shape)
            for i, nm in enumerate(out_names)
        }

    _CACHE["runner"] = (nc, run)
    return run


def kernel(u, W):
    nc = build()
    run = get_runner(nc)
    out = run(make_in_maps(u, W))
    return np.asarray(out["v"][0], dtype=np.float32).reshape(B, OC, OCH)

